# revision 6
# baseline (speedup 1.0000x reference)
"""GNN unpool (gather by clique id + scatter-add by node id) on 8 trn2 cores.

Problem: inputs [B=16, C*NC], node_ids/clique_ids [M], output [B, N*C] where
  pooled = inputs.reshape(B, C, NC)
  out[b, c, node_ids[m]] += pooled[b, c, clique_ids[m]]  for each m

v2 sharding: 2 batch-groups x 4 node-quarters (instead of 8 batch-groups).
The SWDGE (Q7) descriptor-emission cost of dma_gather is ~7.8ns/index and
strictly serialized on GpSimd, so per-core gathered-entry count is the
knob that matters: 25k entries/core (1KB tokens of 512 bc values) instead
of 100k entries/core (256B tokens).

Per-core device algorithm:
  1. load its 8-batch input slice [512, NC] fp32 in 8 pieces,
     PE-transpose -> poolT [NC_pad, 512] bf16 in DRAM
  2. dma_gather 1KB rows of poolT for this quarter's membership entries
     (sorted by node, padded into a canonical cross-core-uniform chunk
     schedule) -> SBUF tokens (entry -> partition e%128, slot e//128)
  3. per 128-entry chunk: one-hot H[entry, rel-node] on DVE via
     is_equal(iota, node - window_base - a_k); PE matmul tokens.T @ H
     accumulates out windows [128bc x 512 node cols] in PSUM; chunk 0 of
     each window streams the full window width with start=True (seeding
     zeros), later chunks accumulate a 256-wide subwindow.
  4. ACT evacuates PSUM -> SBUF staging, DMA -> out [512, NQ] fp32
"""

import math
import sys

import numpy as np

sys.path.insert(0, "/opt/trn_rl_repo")

from concourse import bacc, bass, mybir, tile  # noqa: E402
from concourse.bass_utils import run_bass_kernel_spmd  # noqa: E402
from concourse.masks import make_identity  # noqa: E402

P = 128
N_CORES = 8
N_QUARTERS = 4
N_BGROUPS = 2
SUBW = 256  # H width for non-seed chunks
WINB = 4  # node blocks per psum window
N_GGROUPS = 8  # gather groups

SENT = -4096.0  # nidrel sentinel for padding entries


# ---------------------------------------------------------------- host planning


def _plan(node_ids, clique_ids, NC, N):
    node_ids = np.asarray(node_ids).astype(np.int64)
    clique_ids = np.asarray(clique_ids).astype(np.int64)
    NQ = N // N_QUARTERS  # nodes per quarter
    NCP = math.ceil(NC / P) * P  # padded poolT rows

    nblocks = math.ceil(NQ / P)  # 98
    # windows: groups of WINB node blocks (last window smaller)
    windows = []  # (base_col, ncols)
    b0 = 0
    while b0 < nblocks:
        nb = min(WINB, nblocks - b0)
        windows.append((b0 * P, nb * P))
        b0 += nb
    NW = len(windows)

    # per-quarter sorted entries
    q_snode, q_sclq = [], []
    for q in range(N_QUARTERS):
        mask = (node_ids >= q * NQ) & (node_ids < (q + 1) * NQ)
        nd = node_ids[mask] - q * NQ
        cq = clique_ids[mask]
        order = np.argsort(nd, kind="stable")
        q_snode.append(nd[order])
        q_sclq.append(cq[order])

    # per (quarter, window) counts -> shared chunk allocation
    counts = np.zeros((N_QUARTERS, NW), np.int64)
    for q in range(N_QUARTERS):
        for w, (base, ncols) in enumerate(windows):
            counts[q, w] = int(
                np.count_nonzero(
                    (q_snode[q] >= base) & (q_snode[q] < base + ncols)
                )
            )
    alloc = np.maximum(1, np.ceil(counts.max(axis=0) / P).astype(np.int64))
    U = int(alloc.sum())  # total chunks (shared structure)

    # schedule: per chunk u -> (w, k); subwindow offsets a[u] shared
    sched = []  # (w, k)
    for w in range(NW):
        for k in range(int(alloc[w])):
            sched.append((w, k))

    # per-quarter entry layout into chunks + lo/hi for subwindow placement
    # rel-col arrays per (q, u)
    ent_node = np.full((N_QUARTERS, U, P), -1, np.int64)  # rel col or -1
    ent_clq = np.zeros((N_QUARTERS, U, P), np.int64)
    u = 0
    qpos = np.zeros(N_QUARTERS, np.int64)
    for w, (base, ncols) in enumerate(windows):
        for k in range(int(alloc[w])):
            for q in range(N_QUARTERS):
                pos = qpos[q]
                hi_lim = None
                # entries of this quarter in this window, position bounds
                # consumed sequentially
                remaining = counts[q, w] - k * P
                take = int(max(0, min(P, remaining)))
                if take > 0:
                    sl = slice(pos, pos + take)
                    ent_node[q, u, :take] = q_snode[q][sl] - base
                    ent_clq[q, u, :take] = q_sclq[q][sl]
                    qpos[q] = pos + take
            u += 1
    assert u == U
    for q in range(N_QUARTERS):
        assert qpos[q] == len(q_snode[q])

    # subwindow offsets
    a_off = np.zeros(U, np.int64)
    w_k = np.zeros(U, np.int64)
    wcols = np.zeros(U, np.int64)
    for u, (w, k) in enumerate(sched):
        base, ncols = windows[w]
        wcols[u] = ncols
        if k == 0:
            a_off[u] = 0
            w_k[u] = ncols
            hi = ent_node[:, u][ent_node[:, u] >= 0]
            if hi.size:
                assert hi.max() < ncols
            continue
        w_k[u] = SUBW
        vals = ent_node[:, u][ent_node[:, u] >= 0]
        if vals.size == 0:
            a_off[u] = 0
            continue
        lo, hi = int(vals.min()), int(vals.max())
        assert hi - lo <= SUBW - 1, f"chunk span {hi - lo} too wide (u={u})"
        a = min(max(hi - (SUBW - 1), 0), lo, ncols - SUBW)
        assert a >= 0 and a + SUBW <= ncols and lo >= a and hi < a + SUBW
        a_off[u] = a

    # device tables per quarter
    nidrel = np.full((N_QUARTERS, P, U), SENT, np.float32)
    idxs = np.zeros((N_QUARTERS, U * P), np.int16)
    for q in range(N_QUARTERS):
        for u in range(U):
            nd = ent_node[q, u]
            valid = nd >= 0
            rel = nd - a_off[u]
            nidrel[q, :, u] = np.where(valid, rel, SENT).astype(np.float32)
            idxs[q, u * P : (u + 1) * P] = np.where(
                valid, ent_clq[q, u], 0
            ).astype(np.int16)
    # wrap 16 partitions, replicate to 128
    idx_tbl = np.zeros((N_QUARTERS, P, U * 8), np.int16)
    for q in range(N_QUARTERS):
        wrapped = idxs[q].reshape(-1, 16).T  # [16, U*8]
        idx_tbl[q] = np.tile(wrapped, (8, 1))

    iota = np.tile(
        np.arange(WINB * P, dtype=np.float16)[None, :], (P, 1)
    )

    gsz = math.ceil(U / N_GGROUPS)
    groups = []
    for g in range(N_GGROUPS):
        c0, c1 = g * gsz, min((g + 1) * gsz, U)
        if c0 < c1:
            groups.append((c0, c1))

    return dict(
        NC=NC,
        NCP=NCP,
        N=N,
        NQ=NQ,
        U=U,
        windows=windows,
        sched=sched,
        a_off=a_off,
        w_k=w_k,
        wcols=wcols,
        alloc=alloc,
        nidrel=nidrel,
        idx_tbl=idx_tbl,
        iota=iota,
        gsz=gsz,
        groups=groups,
    )


# ---------------------------------------------------------------- device build


def _build(plan):
    NC = plan["NC"]
    NCP = plan["NCP"]
    NQ = plan["NQ"]
    U = plan["U"]
    windows = plan["windows"]
    sched = plan["sched"]
    a_off = plan["a_off"]
    w_k = plan["w_k"]
    alloc = plan["alloc"]
    gsz = plan["gsz"]
    groups = plan["groups"]

    BC = 512  # bc rows per core (8 batches x 64 ch)
    NT = BC // P  # 4 bc tiles / row groups
    HALF = (NCP // 2)  # 6272 piece width (col half)
    TPH = HALF // P  # 49 transpose tiles per piece

    f32 = mybir.dt.float32
    bf16 = mybir.dt.bfloat16
    f16 = mybir.dt.float16
    i16 = mybir.dt.int16

    nc = bacc.Bacc(None, target_bir_lowering=False)

    pooled_d = nc.dram_tensor("pooled", [BC, NC], f32, kind="ExternalInput")
    idx_d = nc.dram_tensor("idxtbl", [P, U * 8], i16, kind="ExternalInput")
    nidrel_d = nc.dram_tensor("nidrel", [P, U], f32, kind="ExternalInput")
    iota_d = nc.dram_tensor("iotatbl", [P, WINB * P], f16, kind="ExternalInput")
    out_d = nc.dram_tensor("out", [BC, NQ], f32, kind="ExternalOutput")

    with tile.TileContext(nc) as tc:
        with (
            tc.tile_pool(name="dram", bufs=1, space="DRAM") as dramp,
            tc.tile_pool(name="const", bufs=1) as constp,
            tc.tile_pool(name="inp", bufs=2) as inp,
            tc.tile_pool(name="ptst", bufs=2) as ptstp,
            tc.tile_pool(name="tps", bufs=2, space="PSUM") as tpsp,
            tc.tile_pool(name="upool", bufs=2) as upool,
            tc.tile_pool(name="hpool", bufs=4) as hpool,
            tc.tile_pool(name="opsum", bufs=6, space="PSUM") as opsum,
            tc.tile_pool(name="stage", bufs=2) as stagep,
        ):
            ident = constp.tile([P, P], f32)
            make_identity(nc, ident[:])
            iota_t = constp.tile([P, WINB * P], f16)
            nc.sync.dma_start(iota_t[:], iota_d[:])
            nidrel_t = constp.tile([P, U], f32)
            nc.sync.dma_start(nidrel_t[:], nidrel_d[:])
            idx_t = constp.tile([P, U * 8], i16)
            nc.sync.dma_start(idx_t[:], idx_d[:])

            poolT = dramp.tile([NCP, BC], bf16)

            # ---- phase 1: load, transpose, store poolT [NCP, 512] ----
            for s in range(NT):  # bc row group
                for h in range(2):  # column half
                    piece = inp.tile([P, HALF], f32, tag="piece")
                    c0 = h * HALF
                    c1 = min(c0 + HALF, NC)
                    if c1 - c0 < HALF:
                        nc.vector.memset(piece[:, c1 - c0 :], 0.0)
                    nc.sync.dma_start(
                        piece[:, : c1 - c0],
                        pooled_d[s * P : (s + 1) * P, c0:c1],
                    )
                    pst = ptstp.tile([P, TPH, P], bf16, tag="ptst")
                    for t in range(TPH):
                        ps = tpsp.tile([P, P], f32)
                        nc.tensor.transpose(
                            out=ps[:],
                            in_=piece[:, t * P : (t + 1) * P],
                            identity=ident[:],
                        )
                        nc.scalar.copy(pst[:, t, :], ps[:])
                    # strided store: poolT[P*(TPH*h + t) + p, 128s + c]
                    dst = poolT[
                        h * HALF : (h + 1) * HALF, s * P : (s + 1) * P
                    ].rearrange("(t p) c -> p t c", p=P)
                    nc.sync.dma_start(dst, pst[:, :, :])

            # ---- phase 2+3: gather tokens, H one-hots, scatter matmuls ----
            u_tiles = {}

            def ensure_gather(g):
                if g in u_tiles or g >= len(groups):
                    return
                c0, c1 = groups[g]
                nch = c1 - c0
                ut = upool.tile([P, gsz, BC], bf16, tag="utok")
                nidx = nch * P
                nc.gpsimd.dma_gather(
                    out_ap=ut[:, :nch, :],
                    in_ap=poolT[:],
                    idxs_ap=idx_t[:, c0 * 8 : c1 * 8],
                    num_idxs=nidx,
                    num_idxs_reg=nidx,
                    elem_size=BC,
                    single_packet=False,
                )
                u_tiles[g] = ut

            u = 0
            for w, (base, ncols) in enumerate(windows):
                K = int(alloc[w])
                pw = [
                    opsum.tile([P, ncols], f32, tag="ops", name=f"pw{w}_{b}")
                    for b in range(NT)
                ]
                for k in range(K):
                    g = u // gsz
                    ensure_gather(g)
                    ensure_gather(g + 1)
                    wk = int(w_k[u])
                    a = int(a_off[u])
                    ht = hpool.tile([P, WINB * P], bf16, tag="h")
                    nc.vector.tensor_scalar(
                        out=ht[:, :wk],
                        in0=iota_t[:, :wk],
                        scalar1=nidrel_t[:, u : u + 1],
                        scalar2=None,
                        op0=mybir.AluOpType.is_equal,
                    )
                    ut = u_tiles[g]
                    slot = u - g * gsz
                    for b in range(NT):
                        nc.tensor.matmul(
                            out=pw[b][:, a : a + wk],
                            lhsT=ut[:, slot, b * P : (b + 1) * P],
                            rhs=ht[:, :wk],
                            start=(k == 0),
                            stop=(k == K - 1),
                        )
                    u += 1
                # evacuate + store
                wout = min(ncols, NQ - base)
                st = stagep.tile([P, NT, WINB * P], f32, tag="st")
                for b in range(NT):
                    nc.scalar.copy(st[:, b, :ncols], pw[b][:])
                    nc.sync.dma_start(
                        out_d[b * P : (b + 1) * P, base : base + wout],
                        st[:, b, :wout],
                    )
            assert u == U

    nc.finalize()
    return nc


# ---------------------------------------------------------------- entry points

_CACHE = {}


def _get_program(inputs):
    node_ids = np.asarray(inputs["node_ids"])
    clique_ids = np.asarray(inputs["clique_ids"])
    N = int(inputs["nodes"])
    C = int(inputs["n_channels"])
    inputs_arr = np.asarray(inputs["inputs"])
    B, units_dim = inputs_arr.shape
    NC = units_dim // C

    key = (
        B,
        C,
        NC,
        N,
        node_ids.shape[0],
        hash(node_ids.tobytes()),
        hash(clique_ids.tobytes()),
    )
    if key not in _CACHE:
        plan = _plan(node_ids, clique_ids, NC, N)
        nc = _build(plan)
        _CACHE[key] = (plan, nc)
    return _CACHE[key]


def _run(inputs, trace=False):
    inputs_arr = np.asarray(inputs["inputs"]).astype(np.float32)
    N = int(inputs["nodes"])
    C = int(inputs["n_channels"])
    B = inputs_arr.shape[0]
    NC = inputs_arr.shape[1] // C
    NQ = N // N_QUARTERS
    b_per = B // N_BGROUPS  # 8 batches per group

    plan, nc = _get_program(inputs)

    in_maps = []
    for j in range(N_CORES):
        g, q = j // N_QUARTERS, j % N_QUARTERS
        pooled = inputs_arr[g * b_per : (g + 1) * b_per].reshape(
            b_per * C, NC
        )
        in_maps.append(
            {
                "pooled": np.ascontiguousarray(pooled),
                "idxtbl": plan["idx_tbl"][q],
                "nidrel": plan["nidrel"][q],
                "iotatbl": plan["iota"],
            }
        )

    res = run_bass_kernel_spmd(
        nc, in_maps, core_ids=list(range(N_CORES)), trace=trace
    )
    out = np.empty((B, C, N), np.float32)
    for j in range(N_CORES):
        g, q = j // N_QUARTERS, j % N_QUARTERS
        o = res.results[j]["out"]  # [512, NQ]
        out[g * b_per : (g + 1) * b_per, :, q * NQ : (q + 1) * NQ] = (
            o.reshape(b_per, C, NQ)
        )
    return out.reshape(B, C * N), res


def kernel(**inputs) -> np.ndarray:
    out, _ = _run(inputs, trace=False)
    return out


# revision 13
# speedup vs baseline: 1.0604x; 1.0604x over previous
"""GNN unpool (gather by clique id + scatter-add by node id) on 8 trn2 cores.

Problem: inputs [B=16, C*NC], node_ids/clique_ids [M], output [B, N*C] where
  pooled = inputs.reshape(B, C, NC)
  out[b, c, node_ids[m]] += pooled[b, c, clique_ids[m]]  for each m

v2 sharding: 2 batch-groups x 4 node-quarters (instead of 8 batch-groups).
The SWDGE (Q7) descriptor-emission cost of dma_gather is ~7.8ns/index and
strictly serialized on GpSimd, so per-core gathered-entry count is the
knob that matters: 25k entries/core (1KB tokens of 512 bc values) instead
of 100k entries/core (256B tokens).

Per-core device algorithm:
  1. load its 8-batch input slice [512, NC] fp32 in 8 pieces,
     PE-transpose -> poolT [NC_pad, 512] bf16 in DRAM
  2. dma_gather 1KB rows of poolT for this quarter's membership entries
     (sorted by node, padded into a canonical cross-core-uniform chunk
     schedule) -> SBUF tokens (entry -> partition e%128, slot e//128)
  3. per 128-entry chunk: one-hot H[entry, rel-node] on DVE via
     is_equal(iota, node - window_base - a_k); PE matmul tokens.T @ H
     accumulates out windows [128bc x 512 node cols] in PSUM; chunk 0 of
     each window streams the full window width with start=True (seeding
     zeros), later chunks accumulate a 256-wide subwindow.
  4. ACT evacuates PSUM -> SBUF staging, DMA -> out [512, NQ] fp32
"""

import math
import sys

import numpy as np

sys.path.insert(0, "/opt/trn_rl_repo")

from concourse import bacc, bass, mybir, tile  # noqa: E402
from concourse.bass_utils import run_bass_kernel_spmd  # noqa: E402
from concourse.masks import make_identity  # noqa: E402

P = 128
N_CORES = 8
N_QUARTERS = 4
N_BGROUPS = 2
SUBW = 256  # H width for non-seed chunks
WINB = 4  # node blocks per psum window
N_GGROUPS = 8  # gather groups

SENT = -4096.0  # nidrel sentinel for padding entries


# ---------------------------------------------------------------- host planning


def _plan(node_ids, clique_ids, NC, N):
    node_ids = np.asarray(node_ids).astype(np.int64)
    clique_ids = np.asarray(clique_ids).astype(np.int64)
    NQ = N // N_QUARTERS  # nodes per quarter
    NCP = math.ceil(NC / P) * P  # padded poolT rows

    nblocks = math.ceil(NQ / P)  # 98
    # windows: groups of WINB node blocks (last window smaller)
    windows = []  # (base_col, ncols)
    b0 = 0
    while b0 < nblocks:
        nb = min(WINB, nblocks - b0)
        windows.append((b0 * P, nb * P))
        b0 += nb
    NW = len(windows)

    # per-quarter sorted entries
    q_snode, q_sclq = [], []
    for q in range(N_QUARTERS):
        mask = (node_ids >= q * NQ) & (node_ids < (q + 1) * NQ)
        nd = node_ids[mask] - q * NQ
        cq = clique_ids[mask]
        order = np.argsort(nd, kind="stable")
        q_snode.append(nd[order])
        q_sclq.append(cq[order])

    # per (quarter, window) counts -> shared chunk allocation
    counts = np.zeros((N_QUARTERS, NW), np.int64)
    for q in range(N_QUARTERS):
        for w, (base, ncols) in enumerate(windows):
            counts[q, w] = int(
                np.count_nonzero(
                    (q_snode[q] >= base) & (q_snode[q] < base + ncols)
                )
            )
    alloc = np.maximum(1, np.ceil(counts.max(axis=0) / P).astype(np.int64))
    U = int(alloc.sum())  # total chunks (shared structure)

    # schedule: per chunk u -> (w, k); subwindow offsets a[u] shared
    sched = []  # (w, k)
    for w in range(NW):
        for k in range(int(alloc[w])):
            sched.append((w, k))

    # per-quarter entry layout into chunks + lo/hi for subwindow placement
    # rel-col arrays per (q, u)
    ent_node = np.full((N_QUARTERS, U, P), -1, np.int64)  # rel col or -1
    ent_clq = np.zeros((N_QUARTERS, U, P), np.int64)
    u = 0
    qpos = np.zeros(N_QUARTERS, np.int64)
    for w, (base, ncols) in enumerate(windows):
        for k in range(int(alloc[w])):
            for q in range(N_QUARTERS):
                pos = qpos[q]
                hi_lim = None
                # entries of this quarter in this window, position bounds
                # consumed sequentially
                remaining = counts[q, w] - k * P
                take = int(max(0, min(P, remaining)))
                if take > 0:
                    sl = slice(pos, pos + take)
                    ent_node[q, u, :take] = q_snode[q][sl] - base
                    ent_clq[q, u, :take] = q_sclq[q][sl]
                    qpos[q] = pos + take
            u += 1
    assert u == U
    for q in range(N_QUARTERS):
        assert qpos[q] == len(q_snode[q])

    # subwindow offsets + tight widths (multiple of 32)
    a_off = np.zeros(U, np.int64)
    w_k = np.zeros(U, np.int64)
    wcols = np.zeros(U, np.int64)
    for u, (w, k) in enumerate(sched):
        base, ncols = windows[w]
        wcols[u] = ncols
        if k == 0:
            a_off[u] = 0
            w_k[u] = ncols  # seed chunk covers + zeroes the whole window
            hi = ent_node[:, u][ent_node[:, u] >= 0]
            if hi.size:
                assert hi.max() < ncols
            continue
        vals = ent_node[:, u][ent_node[:, u] >= 0]
        if vals.size == 0:
            a_off[u] = 0
            w_k[u] = 32
            continue
        lo, hi = int(vals.min()), int(vals.max())
        span = hi - lo + 1
        assert span <= 512, f"chunk span {span} too wide (u={u})"
        wk = min(max(32 * math.ceil(span / 32), 32), ncols)
        a = min(lo, ncols - wk)
        assert a >= 0 and a + wk <= ncols and lo >= a and hi < a + wk
        a_off[u] = a
        w_k[u] = wk

    # device tables per quarter
    nidrel = np.full((N_QUARTERS, P, U), SENT, np.float32)
    idxs = np.zeros((N_QUARTERS, U * P), np.int16)
    for q in range(N_QUARTERS):
        for u in range(U):
            nd = ent_node[q, u]
            valid = nd >= 0
            rel = nd - a_off[u]
            nidrel[q, :, u] = np.where(valid, rel, SENT).astype(np.float32)
            idxs[q, u * P : (u + 1) * P] = np.where(
                valid, ent_clq[q, u], 0
            ).astype(np.int16)
    # wrap 16 partitions, replicate to 128
    idx_tbl = np.zeros((N_QUARTERS, P, U * 8), np.int16)
    for q in range(N_QUARTERS):
        wrapped = idxs[q].reshape(-1, 16).T  # [16, U*8]
        idx_tbl[q] = np.tile(wrapped, (8, 1))

    iota = np.tile(
        np.arange(WINB * P, dtype=np.float16)[None, :], (P, 1)
    )

    gsz = math.ceil(U / N_GGROUPS)
    groups = []
    for g in range(N_GGROUPS):
        c0, c1 = g * gsz, min((g + 1) * gsz, U)
        if c0 < c1:
            groups.append((c0, c1))

    return dict(
        NC=NC,
        NCP=NCP,
        N=N,
        NQ=NQ,
        U=U,
        windows=windows,
        sched=sched,
        a_off=a_off,
        w_k=w_k,
        wcols=wcols,
        alloc=alloc,
        nidrel=nidrel,
        idx_tbl=idx_tbl,
        iota=iota,
        gsz=gsz,
        groups=groups,
    )


# ---------------------------------------------------------------- device build


def _build(plan):
    NC = plan["NC"]
    NCP = plan["NCP"]
    NQ = plan["NQ"]
    U = plan["U"]
    windows = plan["windows"]
    sched = plan["sched"]
    a_off = plan["a_off"]
    w_k = plan["w_k"]
    alloc = plan["alloc"]
    gsz = plan["gsz"]
    groups = plan["groups"]

    BC = 512  # bc rows per core (8 batches x 64 ch)
    NT = BC // P  # 4 bc tiles / row groups
    NCT = NCP // P  # 98 column tiles
    CGRP = 20  # col tiles per piece
    cgroups = []  # (t0, t1) col tile ranges
    t0 = 0
    while t0 < NCT:
        cgroups.append((t0, min(t0 + CGRP, NCT)))
        t0 += CGRP

    f32 = mybir.dt.float32
    bf16 = mybir.dt.bfloat16
    f16 = mybir.dt.float16
    i16 = mybir.dt.int16

    nc = bacc.Bacc(None, target_bir_lowering=False)

    pooled_d = nc.dram_tensor("pooled", [BC, NC], f32, kind="ExternalInput")
    idx_d = nc.dram_tensor("idxtbl", [P, U * 8], i16, kind="ExternalInput")
    nidrel_d = nc.dram_tensor("nidrel", [P, U], f32, kind="ExternalInput")
    iota_d = nc.dram_tensor("iotatbl", [P, WINB * P], f16, kind="ExternalInput")
    out_d = nc.dram_tensor("out", [BC, NQ], f32, kind="ExternalOutput")

    with tile.TileContext(nc) as tc:
        with (
            tc.tile_pool(name="dram", bufs=1, space="DRAM") as dramp,
            tc.tile_pool(name="const", bufs=1) as constp,
            tc.tile_pool(name="inp", bufs=2) as inp,
            tc.tile_pool(name="ptst", bufs=2) as ptstp,
            tc.tile_pool(name="tps", bufs=2, space="PSUM") as tpsp,
            tc.tile_pool(name="upool", bufs=3) as upool,
            tc.tile_pool(name="hpool", bufs=4) as hpool,
            tc.tile_pool(name="opsum", bufs=6, space="PSUM") as opsum,
            tc.tile_pool(name="stage", bufs=2) as stagep,
        ):
            ident = constp.tile([P, P], bf16)
            make_identity(nc, ident[:])
            iota_t = constp.tile([P, WINB * P], f16)
            nc.sync.dma_start(iota_t[:], iota_d[:])
            nidrel_t = constp.tile([P, U], f32)
            nc.sync.dma_start(nidrel_t[:], nidrel_d[:])
            idx_t = constp.tile([P, U * 8], i16)
            nc.sync.dma_start(idx_t[:], idx_d[:])

            poolT = dramp.tile([NCP, BC], bf16)

            # ---- phase 1: cast-load bf16, transpose, store poolT [NCP, 512] ----
            for s in range(NT):  # bc row group
                for tg0, tg1 in cgroups:  # col tile range
                    ntl = tg1 - tg0
                    w = ntl * P
                    piece = inp.tile([P, CGRP * P], bf16, tag="piece")
                    c0 = tg0 * P
                    c1 = min(tg1 * P, NC)
                    if c1 - c0 < w:
                        nc.vector.memset(piece[:, c1 - c0 : w], 0.0)
                    # SWDGE cast-DMA: fp32 in DRAM -> bf16 in SBUF
                    nc.gpsimd.dma_start(
                        piece[:, : c1 - c0],
                        pooled_d[s * P : (s + 1) * P, c0:c1],
                    )
                    pst = ptstp.tile([P, CGRP, P], bf16, tag="ptst")
                    for t in range(ntl):
                        ps = tpsp.tile([P, P], bf16)
                        nc.tensor.transpose(
                            out=ps[:],
                            in_=piece[:, t * P : (t + 1) * P],
                            identity=ident[:],
                        )
                        nc.scalar.copy(pst[:, t, :], ps[:])
                    # strided store: poolT[P*(tg0 + t) + p, 128s + c]
                    dst = poolT[
                        tg0 * P : tg1 * P, s * P : (s + 1) * P
                    ].rearrange("(t p) c -> p t c", p=P)
                    nc.sync.dma_start(dst, pst[:, :ntl, :])

            # ---- phase 2+3: gather tokens, H one-hots, scatter matmuls ----
            u_tiles = {}

            def ensure_gather(g):
                if g in u_tiles or g >= len(groups):
                    return
                c0, c1 = groups[g]
                nch = c1 - c0
                ut = upool.tile([P, gsz, BC], bf16, tag="utok")
                nidx = nch * P
                nc.gpsimd.dma_gather(
                    out_ap=ut[:, :nch, :],
                    in_ap=poolT[:],
                    idxs_ap=idx_t[:, c0 * 8 : c1 * 8],
                    num_idxs=nidx,
                    num_idxs_reg=nidx,
                    elem_size=BC,
                    single_packet=False,
                )
                u_tiles[g] = ut

            u = 0
            for w, (base, ncols) in enumerate(windows):
                K = int(alloc[w])
                pw = [
                    opsum.tile([P, ncols], f32, tag="ops", name=f"pw{w}_{b}")
                    for b in range(NT)
                ]
                for k in range(K):
                    g = u // gsz
                    ensure_gather(g)
                    ensure_gather(g + 1)
                    ensure_gather(g + 2)
                    wk = int(w_k[u])
                    a = int(a_off[u])
                    ht = hpool.tile([P, WINB * P], bf16, tag="h")
                    nc.vector.tensor_scalar(
                        out=ht[:, :wk],
                        in0=iota_t[:, :wk],
                        scalar1=nidrel_t[:, u : u + 1],
                        scalar2=None,
                        op0=mybir.AluOpType.is_equal,
                    )
                    ut = u_tiles[g]
                    slot = u - g * gsz
                    for b in range(NT):
                        nc.tensor.matmul(
                            out=pw[b][:, a : a + wk],
                            lhsT=ut[:, slot, b * P : (b + 1) * P],
                            rhs=ht[:, :wk],
                            start=(k == 0),
                            stop=(k == K - 1),
                        )
                    u += 1
                # evacuate + store
                wout = min(ncols, NQ - base)
                st = stagep.tile([P, NT, WINB * P], f32, tag="st")
                for b in range(NT):
                    nc.scalar.copy(st[:, b, :ncols], pw[b][:])
                    nc.sync.dma_start(
                        out_d[b * P : (b + 1) * P, base : base + wout],
                        st[:, b, :wout],
                    )
            assert u == U

    nc.finalize()
    return nc


# ---------------------------------------------------------------- entry points

_CACHE = {}


def _get_program(inputs):
    node_ids = np.asarray(inputs["node_ids"])
    clique_ids = np.asarray(inputs["clique_ids"])
    N = int(inputs["nodes"])
    C = int(inputs["n_channels"])
    inputs_arr = np.asarray(inputs["inputs"])
    B, units_dim = inputs_arr.shape
    NC = units_dim // C

    key = (
        B,
        C,
        NC,
        N,
        node_ids.shape[0],
        hash(node_ids.tobytes()),
        hash(clique_ids.tobytes()),
    )
    if key not in _CACHE:
        plan = _plan(node_ids, clique_ids, NC, N)
        nc = _build(plan)
        _CACHE[key] = (plan, nc)
    return _CACHE[key]


def _run(inputs, trace=False):
    inputs_arr = np.asarray(inputs["inputs"]).astype(np.float32)
    N = int(inputs["nodes"])
    C = int(inputs["n_channels"])
    B = inputs_arr.shape[0]
    NC = inputs_arr.shape[1] // C
    NQ = N // N_QUARTERS
    b_per = B // N_BGROUPS  # 8 batches per group

    plan, nc = _get_program(inputs)

    in_maps = []
    for j in range(N_CORES):
        g, q = j // N_QUARTERS, j % N_QUARTERS
        pooled = inputs_arr[g * b_per : (g + 1) * b_per].reshape(
            b_per * C, NC
        )
        in_maps.append(
            {
                "pooled": np.ascontiguousarray(pooled),
                "idxtbl": plan["idx_tbl"][q],
                "nidrel": plan["nidrel"][q],
                "iotatbl": plan["iota"],
            }
        )

    res = run_bass_kernel_spmd(
        nc, in_maps, core_ids=list(range(N_CORES)), trace=trace
    )
    out = np.empty((B, C, N), np.float32)
    for j in range(N_CORES):
        g, q = j // N_QUARTERS, j % N_QUARTERS
        o = res.results[j]["out"]  # [512, NQ]
        out[g * b_per : (g + 1) * b_per, :, q * NQ : (q + 1) * NQ] = (
            o.reshape(b_per, C, NQ)
        )
    return out.reshape(B, C * N), res


def kernel(**inputs) -> np.ndarray:
    out, _ = _run(inputs, trace=False)
    return out


# revision 19
# speedup vs baseline: 1.3645x; 1.2868x over previous
"""GNN unpool (gather by clique id + scatter-add by node id) on 8 trn2 cores.

Problem: inputs [B=16, C*NC], node_ids/clique_ids [M], output [B, N*C] where
  pooled = inputs.reshape(B, C, NC)
  out[b, c, node_ids[m]] += pooled[b, c, clique_ids[m]]  for each m

v2 sharding: 2 batch-groups x 4 node-quarters (instead of 8 batch-groups).
The SWDGE (Q7) descriptor-emission cost of dma_gather is ~7.8ns/index and
strictly serialized on GpSimd, so per-core gathered-entry count is the
knob that matters: 25k entries/core (1KB tokens of 512 bc values) instead
of 100k entries/core (256B tokens).

Per-core device algorithm:
  1. load its 8-batch input slice [512, NC] fp32 in 8 pieces,
     PE-transpose -> poolT [NC_pad, 512] bf16 in DRAM
  2. dma_gather 1KB rows of poolT for this quarter's membership entries
     (sorted by node, padded into a canonical cross-core-uniform chunk
     schedule) -> SBUF tokens (entry -> partition e%128, slot e//128)
  3. per 128-entry chunk: one-hot H[entry, rel-node] on DVE via
     is_equal(iota, node - window_base - a_k); PE matmul tokens.T @ H
     accumulates out windows [128bc x 512 node cols] in PSUM; chunk 0 of
     each window streams the full window width with start=True (seeding
     zeros), later chunks accumulate a 256-wide subwindow.
  4. ACT evacuates PSUM -> SBUF staging, DMA -> out [512, NQ] fp32
"""

import math
import sys

import numpy as np

sys.path.insert(0, "/opt/trn_rl_repo")

from concourse import bacc, bass, mybir, tile  # noqa: E402
from concourse.bass_utils import run_bass_kernel_spmd  # noqa: E402
from concourse.masks import make_identity  # noqa: E402

P = 128
N_CORES = 8
N_QUARTERS = 4
N_BGROUPS = 2
SUBW = 256  # H width for non-seed chunks
WINB = 4  # node blocks per psum window
N_GGROUPS = 8  # gather groups

SENT = -4096.0  # nidrel sentinel for padding entries


# ---------------------------------------------------------------- host planning


def _plan(node_ids, clique_ids, NC, N):
    node_ids = np.asarray(node_ids).astype(np.int64)
    clique_ids = np.asarray(clique_ids).astype(np.int64)
    NQ = N // N_QUARTERS  # nodes per quarter
    NCP = math.ceil(NC / P) * P  # padded poolT rows

    nblocks = math.ceil(NQ / P)  # 98
    # windows: groups of WINB node blocks (last window smaller)
    windows = []  # (base_col, ncols)
    b0 = 0
    while b0 < nblocks:
        nb = min(WINB, nblocks - b0)
        windows.append((b0 * P, nb * P))
        b0 += nb
    NW = len(windows)

    # per-quarter sorted entries
    q_snode, q_sclq = [], []
    for q in range(N_QUARTERS):
        mask = (node_ids >= q * NQ) & (node_ids < (q + 1) * NQ)
        nd = node_ids[mask] - q * NQ
        cq = clique_ids[mask]
        order = np.argsort(nd, kind="stable")
        q_snode.append(nd[order])
        q_sclq.append(cq[order])

    # per (quarter, window) counts -> shared chunk allocation
    counts = np.zeros((N_QUARTERS, NW), np.int64)
    for q in range(N_QUARTERS):
        for w, (base, ncols) in enumerate(windows):
            counts[q, w] = int(
                np.count_nonzero(
                    (q_snode[q] >= base) & (q_snode[q] < base + ncols)
                )
            )
    alloc = np.maximum(1, np.ceil(counts.max(axis=0) / P).astype(np.int64))
    U = int(alloc.sum())  # total chunks (shared structure)

    # schedule: per chunk u -> (w, k); subwindow offsets a[u] shared
    sched = []  # (w, k)
    for w in range(NW):
        for k in range(int(alloc[w])):
            sched.append((w, k))

    # per-quarter entry layout into chunks + lo/hi for subwindow placement
    # rel-col arrays per (q, u)
    ent_node = np.full((N_QUARTERS, U, P), -1, np.int64)  # rel col or -1
    ent_clq = np.zeros((N_QUARTERS, U, P), np.int64)
    u = 0
    qpos = np.zeros(N_QUARTERS, np.int64)
    for w, (base, ncols) in enumerate(windows):
        for k in range(int(alloc[w])):
            for q in range(N_QUARTERS):
                pos = qpos[q]
                hi_lim = None
                # entries of this quarter in this window, position bounds
                # consumed sequentially
                remaining = counts[q, w] - k * P
                take = int(max(0, min(P, remaining)))
                if take > 0:
                    sl = slice(pos, pos + take)
                    ent_node[q, u, :take] = q_snode[q][sl] - base
                    ent_clq[q, u, :take] = q_sclq[q][sl]
                    qpos[q] = pos + take
            u += 1
    assert u == U
    for q in range(N_QUARTERS):
        assert qpos[q] == len(q_snode[q])

    # subwindow offsets + tight widths (multiple of 32)
    a_off = np.zeros(U, np.int64)
    w_k = np.zeros(U, np.int64)
    wcols = np.zeros(U, np.int64)
    for u, (w, k) in enumerate(sched):
        base, ncols = windows[w]
        wcols[u] = ncols
        if k == 0:
            a_off[u] = 0
            w_k[u] = ncols  # seed chunk covers + zeroes the whole window
            hi = ent_node[:, u][ent_node[:, u] >= 0]
            if hi.size:
                assert hi.max() < ncols
            continue
        vals = ent_node[:, u][ent_node[:, u] >= 0]
        if vals.size == 0:
            a_off[u] = 0
            w_k[u] = 32
            continue
        lo, hi = int(vals.min()), int(vals.max())
        span = hi - lo + 1
        assert span <= 512, f"chunk span {span} too wide (u={u})"
        wk = min(max(32 * math.ceil(span / 32), 32), ncols)
        a = min(lo, ncols - wk)
        assert a >= 0 and a + wk <= ncols and lo >= a and hi < a + wk
        a_off[u] = a
        w_k[u] = wk

    # device tables per quarter
    nidrel = np.full((N_QUARTERS, P, U), SENT, np.float32)
    idxs = np.zeros((N_QUARTERS, U * P), np.int16)
    for q in range(N_QUARTERS):
        for u in range(U):
            nd = ent_node[q, u]
            valid = nd >= 0
            rel = nd - a_off[u]
            nidrel[q, :, u] = np.where(valid, rel, SENT).astype(np.float32)
            idxs[q, u * P : (u + 1) * P] = np.where(
                valid, ent_clq[q, u], 0
            ).astype(np.int16)
    # wrap 16 partitions, replicate to 128
    idx_tbl = np.zeros((N_QUARTERS, P, U * 8), np.int16)
    for q in range(N_QUARTERS):
        wrapped = idxs[q].reshape(-1, 16).T  # [16, U*8]
        idx_tbl[q] = np.tile(wrapped, (8, 1))

    iota = np.tile(
        np.arange(WINB * P, dtype=np.float16)[None, :], (P, 1)
    )

    # gather groups: ~28 chunks each, tapered tail so the last drain is short
    gsz = 28
    groups = []
    c0 = 0
    while U - c0 > 24:
        groups.append((c0, min(c0 + gsz, U)))
        c0 = groups[-1][1]
    rem = U - c0
    if rem > 0:
        cut = c0 + (rem * 2 + 2) // 3
        groups.append((c0, cut))
        if cut < U:
            groups.append((cut, U))

    return dict(
        NC=NC,
        NCP=NCP,
        N=N,
        NQ=NQ,
        U=U,
        windows=windows,
        sched=sched,
        a_off=a_off,
        w_k=w_k,
        wcols=wcols,
        alloc=alloc,
        nidrel=nidrel,
        idx_tbl=idx_tbl,
        iota=iota,
        gsz=gsz,
        groups=groups,
    )


# ---------------------------------------------------------------- device build


def _build(plan):
    NC = plan["NC"]
    NCP = plan["NCP"]
    NQ = plan["NQ"]
    U = plan["U"]
    windows = plan["windows"]
    sched = plan["sched"]
    a_off = plan["a_off"]
    w_k = plan["w_k"]
    alloc = plan["alloc"]
    gsz = plan["gsz"]
    groups = plan["groups"]

    BC = 512  # bc rows per core (8 batches x 64 ch)
    NT = BC // P  # 4 bc tiles / row groups
    NCT = NCP // P  # 98 column tiles
    CGRP = 20  # col tiles per piece
    cgroups = []  # (t0, t1) col tile ranges
    t0 = 0
    while t0 < NCT:
        cgroups.append((t0, min(t0 + CGRP, NCT)))
        t0 += CGRP

    f32 = mybir.dt.float32
    bf16 = mybir.dt.bfloat16
    f16 = mybir.dt.float16
    i16 = mybir.dt.int16

    nc = bacc.Bacc(None, target_bir_lowering=False, num_swdge_queues=2)

    pooled_d = nc.dram_tensor("pooled", [BC, NC], f32, kind="ExternalInput")
    idx_d = nc.dram_tensor("idxtbl", [P, U * 8], i16, kind="ExternalInput")
    nidrel_d = nc.dram_tensor("nidrel", [P, U], f32, kind="ExternalInput")
    iota_d = nc.dram_tensor("iotatbl", [P, WINB * P], f16, kind="ExternalInput")
    out_d = nc.dram_tensor("out", [BC, NQ], f32, kind="ExternalOutput")

    with tile.TileContext(nc) as tc:
        with (
            tc.tile_pool(name="dram", bufs=1, space="DRAM") as dramp,
            tc.tile_pool(name="const", bufs=1) as constp,
            tc.tile_pool(name="inp", bufs=2) as inp,
            tc.tile_pool(name="ptst", bufs=2) as ptstp,
            tc.tile_pool(name="tps", bufs=2, space="PSUM") as tpsp,
            tc.tile_pool(name="upool", bufs=3) as upool,
            tc.tile_pool(name="hpool", bufs=4) as hpool,
            tc.tile_pool(name="opsum", bufs=6, space="PSUM") as opsum,
            tc.tile_pool(name="stage", bufs=2) as stagep,
        ):
            ident = constp.tile([P, P], bf16)
            make_identity(nc, ident[:])
            iota_t = constp.tile([P, WINB * P], f16)
            nc.sync.dma_start(iota_t[:], iota_d[:])
            nidrel_t = constp.tile([P, U], f32)
            nc.sync.dma_start(nidrel_t[:], nidrel_d[:])
            idx_t = constp.tile([P, U * 8], i16)
            nc.sync.dma_start(idx_t[:], idx_d[:])

            poolT = dramp.tile([NCP, BC], bf16)

            # ---- phase 1: cast-load bf16, transpose, store poolT [NCP, 512] ----
            for s in range(NT):  # bc row group
                for tg0, tg1 in cgroups:  # col tile range
                    ntl = tg1 - tg0
                    w = ntl * P
                    piece = inp.tile([P, CGRP * P], bf16, tag="piece")
                    c0 = tg0 * P
                    c1 = min(tg1 * P, NC)
                    if c1 - c0 < w:
                        nc.vector.memset(piece[:, c1 - c0 : w], 0.0)
                    # SWDGE cast-DMA: fp32 in DRAM -> bf16 in SBUF
                    nc.gpsimd.dma_start(
                        piece[:, : c1 - c0],
                        pooled_d[s * P : (s + 1) * P, c0:c1],
                    )
                    pst = ptstp.tile([P, CGRP, P], bf16, tag="ptst")
                    for t0b in range(0, ntl, 4):
                        t1b = min(t0b + 4, ntl)
                        ps = tpsp.tile([P, 4 * P], bf16)
                        for t in range(t0b, t1b):
                            nc.tensor.transpose(
                                out=ps[:, (t - t0b) * P : (t - t0b + 1) * P],
                                in_=piece[:, t * P : (t + 1) * P],
                                identity=ident[:],
                            )
                        nc.scalar.copy(
                            pst[:, t0b:t1b, :], ps[:, : (t1b - t0b) * P]
                        )
                    # strided store: poolT[P*(tg0 + t) + p, 128s + c]
                    dst = poolT[
                        tg0 * P : tg1 * P, s * P : (s + 1) * P
                    ].rearrange("(t p) c -> p t c", p=P)
                    nc.sync.dma_start(dst, pst[:, :ntl, :])

            # ---- phase 2+3: gather tokens, H one-hots, scatter matmuls ----
            u_tiles = {}
            u2g = np.zeros(U, np.int64)
            for g, (c0, c1) in enumerate(groups):
                u2g[c0:c1] = g

            def ensure_gather(g):
                if g in u_tiles or g >= len(groups):
                    return
                c0, c1 = groups[g]
                nch = c1 - c0
                ut = upool.tile([P, gsz, BC], bf16, tag="utok")
                nidx = nch * P
                nc.gpsimd.dma_gather(
                    out_ap=ut[:, :nch, :],
                    in_ap=poolT[:],
                    idxs_ap=idx_t[:, c0 * 8 : c1 * 8],
                    num_idxs=nidx,
                    num_idxs_reg=nidx,
                    elem_size=BC,
                    single_packet=False,
                    queue_num=g % 2,
                )
                u_tiles[g] = ut

            u = 0
            for w, (base, ncols) in enumerate(windows):
                K = int(alloc[w])
                pw = [
                    opsum.tile([P, ncols], f32, tag="ops", name=f"pw{w}_{b}")
                    for b in range(NT)
                ]
                for k in range(K):
                    g = int(u2g[u])
                    ensure_gather(g)
                    ensure_gather(g + 1)
                    ensure_gather(g + 2)
                    wk = int(w_k[u])
                    a = int(a_off[u])
                    ht = hpool.tile([P, WINB * P], bf16, tag="h")
                    nc.vector.tensor_scalar(
                        out=ht[:, :wk],
                        in0=iota_t[:, :wk],
                        scalar1=nidrel_t[:, u : u + 1],
                        scalar2=None,
                        op0=mybir.AluOpType.is_equal,
                    )
                    ut = u_tiles[g]
                    slot = u - groups[g][0]
                    for b in range(NT):
                        nc.tensor.matmul(
                            out=pw[b][:, a : a + wk],
                            lhsT=ut[:, slot, b * P : (b + 1) * P],
                            rhs=ht[:, :wk],
                            start=(k == 0),
                            stop=(k == K - 1),
                        )
                    u += 1
                # evacuate + store
                wout = min(ncols, NQ - base)
                st = stagep.tile([P, NT, WINB * P], f32, tag="st")
                for b in range(NT):
                    nc.scalar.copy(st[:, b, :ncols], pw[b][:])
                    nc.sync.dma_start(
                        out_d[b * P : (b + 1) * P, base : base + wout],
                        st[:, b, :wout],
                    )
            assert u == U

    nc.finalize()
    return nc


# ---------------------------------------------------------------- entry points

_CACHE = {}


def _get_program(inputs):
    node_ids = np.asarray(inputs["node_ids"])
    clique_ids = np.asarray(inputs["clique_ids"])
    N = int(inputs["nodes"])
    C = int(inputs["n_channels"])
    inputs_arr = np.asarray(inputs["inputs"])
    B, units_dim = inputs_arr.shape
    NC = units_dim // C

    key = (
        B,
        C,
        NC,
        N,
        node_ids.shape[0],
        hash(node_ids.tobytes()),
        hash(clique_ids.tobytes()),
    )
    if key not in _CACHE:
        plan = _plan(node_ids, clique_ids, NC, N)
        nc = _build(plan)
        _CACHE[key] = (plan, nc)
    return _CACHE[key]


def _run(inputs, trace=False):
    inputs_arr = np.asarray(inputs["inputs"]).astype(np.float32)
    N = int(inputs["nodes"])
    C = int(inputs["n_channels"])
    B = inputs_arr.shape[0]
    NC = inputs_arr.shape[1] // C
    NQ = N // N_QUARTERS
    b_per = B // N_BGROUPS  # 8 batches per group

    plan, nc = _get_program(inputs)

    in_maps = []
    for j in range(N_CORES):
        g, q = j // N_QUARTERS, j % N_QUARTERS
        pooled = inputs_arr[g * b_per : (g + 1) * b_per].reshape(
            b_per * C, NC
        )
        in_maps.append(
            {
                "pooled": np.ascontiguousarray(pooled),
                "idxtbl": plan["idx_tbl"][q],
                "nidrel": plan["nidrel"][q],
                "iotatbl": plan["iota"],
            }
        )

    res = run_bass_kernel_spmd(
        nc, in_maps, core_ids=list(range(N_CORES)), trace=trace
    )
    out = np.empty((B, C, N), np.float32)
    for j in range(N_CORES):
        g, q = j // N_QUARTERS, j % N_QUARTERS
        o = res.results[j]["out"]  # [512, NQ]
        out[g * b_per : (g + 1) * b_per, :, q * NQ : (q + 1) * NQ] = (
            o.reshape(b_per, C, NQ)
        )
    return out.reshape(B, C * N), res


def kernel(**inputs) -> np.ndarray:
    out, _ = _run(inputs, trace=False)
    return out
